# revision 1
# baseline (speedup 1.0000x reference)
"""OIM loss with circular queue — Trainium2 Bass kernel (8 NeuronCores).

Strategy
--------
The output is a scalar:  loss = mean_b [ logsumexp_{q in good}(30*cos(x_b, e_q))
                                         - 30*cos(x_b, e_{xe_b}) ]
where e is the circular queue after the (sequential, data-dependent) update.

The queue update only *moves integer labels around* plus writes U=256
normalized per-pid mean embeddings into a contiguous window of slots.  All the
integer bookkeeping (which slots are invalidated, which slot each batch row
targets) is done on the host; every FLOP-heavy part runs on the 8 cores:

  - per-pid masked means  (one-hot mask matmul,  [U,B]x[B,D])
  - row normalization of inputs and means
  - the big logits matmul [B,Q//8,D] per core (float32r, full PE rate)
    fused with exp (ACT: exp(30*s - M), M a safe upper bound of the row max)
    and the masked row-sum (DVE tensor_tensor_reduce with the `good` mask)
  - target cosines via a small [B,U] matmul + one-hot gather

Sharding: emb_cq is sharded over Q (2048 rows/core, tensor-parallel); the
batch-side preprocessing is replicated (it is ~2% of the FLOPs).  Each core
returns partial sums of exp(logit - M) over its Q-shard plus the target
cosines; the host adds the 8 partials (the "allreduce"), takes log and means.
"""

import os
import sys

import numpy as np

for _p in ("/opt/trn_rl_repo", "/root/.axon_site/_ro/trn_rl_repo"):
    if os.path.isdir(_p) and _p not in sys.path:
        sys.path.insert(0, _p)

B, D, Q, U = 4096, 512, 16384, 256
N_CORES = 8
QS = Q // N_CORES          # queue rows per core
OIM_SCALAR = 30.0
IGNORE = -1
MT = B // 128              # 32 b-tiles
QT = QS // 128             # 16 q-tiles per core
KD = D // 128              # 4 contraction chunks
NQ = QS // 512             # 4 matmul n-chunks per core
UT = U // 128              # 2 u-tiles

_PROG_CACHE = {}


def _build_program(M: float, work_bufs=4, psm_bufs=2, kd_outer=False, pst_bufs=4, small_bufs=6, exp_bufs=6, tl_bufs=4):
    """Emit + schedule + compile the (SPMD, identical on all cores) program."""
    import concourse.bacc as bacc
    import concourse.bass as bass
    import concourse.tile as tile
    from concourse import mybir
    from concourse.masks import make_identity

    f32 = mybir.dt.float32
    f32r = mybir.dt.float32r
    AF = mybir.ActivationFunctionType
    OP = mybir.AluOpType

    nc = bacc.Bacc("TRN2", target_bir_lowering=False, debug=False,
                   num_devices=N_CORES)

    x_d = nc.dram_tensor("x", [B, D], f32, kind="ExternalInput").ap()
    emb_d = nc.dram_tensor("emb", [QS, D], f32, kind="ExternalInput").ap()
    labf_d = nc.dram_tensor("labf", [128, MT], f32, kind="ExternalInput").ap()
    uniqf_d = nc.dram_tensor("uniqf", [128, U], f32, kind="ExternalInput").ap()
    cnts_d = nc.dram_tensor("cnts", [128, UT], f32, kind="ExternalInput").ap()
    widx_d = nc.dram_tensor("widx", [128, MT], f32, kind="ExternalInput").ap()
    iota_d = nc.dram_tensor("iota", [128, U], f32, kind="ExternalInput").ap()
    gkeep_d = nc.dram_tensor("gkeep", [128, QS], f32, kind="ExternalInput").ap()
    wkeep_d = nc.dram_tensor("wkeep", [128, QT], f32, kind="ExternalInput").ap()
    oht_d = nc.dram_tensor("oht", [128, UT, QS], f32, kind="ExternalInput").ap()
    sume_d = nc.dram_tensor("sume", [128, MT], f32, kind="ExternalOutput").ap()
    tco_d = nc.dram_tensor("tco", [128, MT], f32, kind="ExternalOutput").ap()
    tick_d = nc.dram_tensor("tick", [128, 4], f32, kind="ExternalInput").ap()
    tock_d = nc.dram_tensor("tock", [128, 4], f32, kind="ExternalOutput").ap()

    with tile.TileContext(nc) as tc:
        with (
            tc.tile_pool(name="singles", bufs=1) as singles,
            tc.tile_pool(name="work", bufs=work_bufs) as work,
            tc.tile_pool(name="small", bufs=small_bufs) as small,
            tc.tile_pool(name="psum_t", bufs=pst_bufs, space="PSUM") as psum_t,
        ):
            # ---------------- constants / small inputs ----------------
            ident = singles.tile([128, 128], f32)
            make_identity(nc, ident)

            labs = singles.tile([128, MT], f32)
            nc.sync.dma_start(out=labs, in_=labf_d)
            widx = singles.tile([128, MT], f32)
            nc.sync.dma_start(out=widx, in_=widx_d)
            wkp = singles.tile([128, QT], f32)
            nc.sync.dma_start(out=wkp, in_=wkeep_d)
            cnts = singles.tile([128, UT], f32)
            nc.sync.dma_start(out=cnts, in_=cnts_d)
            uniqb = singles.tile([128, U], f32)
            nc.sync.dma_start(out=uniqb, in_=uniqf_d)
            iotab = singles.tile([128, U], f32)
            nc.sync.dma_start(out=iotab, in_=iota_d)
            keepg = singles.tile([128, QS], f32)
            nc.sync.dma_start(out=keepg, in_=gkeep_d)
            oht = singles.tile([128, UT, QS], f32r)
            nc.sync.dma_start(out=oht, in_=oht_d.bitcast(f32r))

            rcnt = singles.tile([128, UT], f32)
            nc.vector.reciprocal(rcnt, cnts)
            biasM = singles.tile([128, 1], f32)
            nc.vector.memset(biasM, -M)

            # resident big tensors
            xn_all = singles.tile([128, MT, D], f32)     # normalized inputs (b-major)
            embT = singles.tile([128, KD, QS], f32r)     # blended emb, d-major
            uembT = singles.tile([128, KD, U], f32r)     # uniq means, d-major
            uemb_n = singles.tile([128, UT, D], f32r)    # uniq means, u-major
            ssb = singles.tile([128, MT], f32)           # sum-exp out collector
            tsb = singles.tile([128, MT], f32)           # target-cos out collector

            # ---------------- phase 1+2: masked means + normalize ----------
            with tc.tile_pool(name="psum_u", bufs=1, space="PSUM") as psum_u:
                ps_u = [psum_u.tile([128, D], f32, tag=f"uniq{mu}",
                                    name=f"ps_u{mu}") for mu in range(UT)]
                for i in range(MT):
                    x_raw = work.tile([128, D], f32r, tag="x_raw")
                    nc.sync.dma_start(out=x_raw,
                                      in_=x_d[i * 128:(i + 1) * 128, :].bitcast(f32r))
                    x_f = x_raw.bitcast(f32)

                    # mask[b, u] = (uniq[u] == labels[b])
                    mt_ = work.tile([128, U], f32r, tag="maskr")
                    nc.vector.tensor_scalar(out=mt_, in0=uniqb,
                                            scalar1=labs[:, i:i + 1], scalar2=None,
                                            op0=OP.is_equal)
                    for mu in range(UT):
                        nc.tensor.matmul(ps_u[mu],
                                         mt_[:, mu * 128:(mu + 1) * 128],
                                         x_raw, start=(i == 0),
                                         stop=(i == MT - 1))

                    # row-normalize x
                    sq = work.tile([128, D], f32, tag="sq")
                    ssq = small.tile([128, 1], f32, tag="ssq")
                    nc.vector.scalar_tensor_tensor(out=sq, in0=x_f, scalar=1.0,
                                                   in1=x_f, op0=OP.mult,
                                                   op1=OP.mult, accum_out=ssq)
                    nrm = small.tile([128, 1], f32, tag="nrm")
                    nc.scalar.activation(out=nrm, in_=ssq, func=AF.Sqrt)
                    nc.vector.tensor_scalar_max(out=nrm, in0=nrm, scalar1=1e-12)
                    rin = small.tile([128, 1], f32, tag="rin")
                    nc.vector.reciprocal(rin, nrm)
                    nc.vector.tensor_scalar_mul(out=xn_all[:, i, :], in0=x_f,
                                                scalar1=rin)

                # finish uniq means: mean, normalize, transpose to d-major
                for mu in range(UT):
                    ue = uemb_n[:, mu, :]
                    nc.vector.tensor_scalar_mul(out=ue, in0=ps_u[mu],
                                                scalar1=rcnt[:, mu:mu + 1])
                    sq2 = work.tile([128, D], f32, tag="sq")
                    ssq2 = small.tile([128, 1], f32, tag="ssq")
                    ue_f = ue.bitcast(f32)
                    nc.vector.scalar_tensor_tensor(out=sq2, in0=ue_f, scalar=1.0,
                                                   in1=ue_f, op0=OP.mult,
                                                   op1=OP.mult, accum_out=ssq2)
                    nrm2 = small.tile([128, 1], f32, tag="nrm")
                    nc.scalar.activation(out=nrm2, in_=ssq2, func=AF.Sqrt)
                    nc.vector.tensor_scalar_max(out=nrm2, in0=nrm2, scalar1=1e-12)
                    rin2 = small.tile([128, 1], f32, tag="rin")
                    nc.vector.reciprocal(rin2, nrm2)
                    nc.vector.tensor_scalar_mul(out=ue, in0=ue_f, scalar1=rin2)
                    for kd in range(KD):
                        pst = psum_t.tile([128, 128], f32, tag="pst")
                        nc.tensor.transpose(
                            pst,
                            uemb_n[:, mu, kd * 128:(kd + 1) * 128].bitcast(f32),
                            ident)
                        nc.scalar.copy(out=uembT[:, kd, mu * 128:(mu + 1) * 128],
                                       in_=pst)

            # ---------------- phase 3: blend queue window + transpose ------
            with tc.tile_pool(name="psum_b", bufs=2, space="PSUM") as psum_b:
                for t in range(QT):
                    e_raw = work.tile([128, D], f32, tag="e_raw")
                    nc.sync.dma_start(out=e_raw,
                                      in_=emb_d[t * 128:(t + 1) * 128, :])
                    eff = work.tile([128, D], f32, tag="eff")
                    # zero the window rows ...
                    nc.vector.tensor_scalar_mul(out=eff, in0=e_raw,
                                                scalar1=wkp[:, t:t + 1])
                    # ... and add one-hot @ uniq_means
                    psb = psum_b.tile([128, D], f32, tag="psb")
                    for ku in range(UT):
                        nc.tensor.matmul(psb,
                                         oht[:, ku, t * 128:(t + 1) * 128],
                                         uemb_n[:, ku, :],
                                         start=(ku == 0), stop=(ku == UT - 1))
                    nc.vector.tensor_add(out=eff, in0=eff, in1=psb)
                    for kd in range(KD):
                        pst = psum_t.tile([128, 128], f32, tag="pst")
                        nc.tensor.transpose(pst, eff[:, kd * 128:(kd + 1) * 128],
                                            ident)
                        nc.scalar.copy(out=embT[:, kd, t * 128:(t + 1) * 128],
                                       in_=pst)

            # ---------------- phase 4: logits + fused LSE ----------------
            with (
                tc.tile_pool(name="psum_s", bufs=2, space="PSUM") as psum_s,
                tc.tile_pool(name="psum_m", bufs=psm_bufs, space="PSUM") as psum_m,
            ):
                for m in range(MT):
                    tl = work.tile([128, D], f32r, tag="lhsT", bufs=tl_bufs)
                    for kd in range(KD):
                        pst = psum_t.tile([128, 128], f32, tag="pst")
                        nc.tensor.transpose(
                            pst, xn_all[:, m, kd * 128:(kd + 1) * 128], ident)
                        nc.scalar.copy(out=tl[:, kd * 128:(kd + 1) * 128], in_=pst)

                    # target cosines: S2[b, u] then one-hot gather along u
                    pss = psum_s.tile([128, U], f32, tag="pss")
                    for kd in range(KD):
                        nc.tensor.matmul(pss, tl[:, kd * 128:(kd + 1) * 128],
                                         uembT[:, kd, :],
                                         start=(kd == 0), stop=(kd == KD - 1))
                    scr_u = work.tile([128, U], f32, tag="mask")
                    nc.vector.scalar_tensor_tensor(out=scr_u, in0=iotab,
                                                   scalar=widx[:, m:m + 1],
                                                   in1=pss,
                                                   op0=OP.is_equal, op1=OP.mult,
                                                   accum_out=tsb[:, m:m + 1])

                    # big matmul over this core's Q-shard, fused exp+masked sum
                    acc4 = small.tile([128, NQ], f32, tag="acc4")
                    if kd_outer:
                        psms = [psum_m.tile([128, 512], f32, tag=f"psm{n}",
                                            name=f"psm_{m}_{n}") for n in range(NQ)]
                        for kd in range(KD):
                            for n in range(NQ):
                                nc.tensor.matmul(
                                    psms[n], tl[:, kd * 128:(kd + 1) * 128],
                                    embT[:, kd, n * 512:(n + 1) * 512],
                                    start=(kd == 0), stop=(kd == KD - 1))
                        for n in range(NQ):
                            expt = work.tile([128, 512], f32, tag="expt", bufs=exp_bufs)
                            nc.scalar.activation(out=expt, in_=psms[n], func=AF.Exp,
                                                 bias=biasM, scale=OIM_SCALAR)
                            scr = work.tile([128, 512], f32, tag="scr", bufs=exp_bufs)
                            nc.vector.scalar_tensor_tensor(
                                out=scr, in0=expt, scalar=1.0,
                                in1=keepg[:, n * 512:(n + 1) * 512],
                                op0=OP.mult, op1=OP.mult,
                                accum_out=acc4[:, n:n + 1])
                    else:
                        for n in range(NQ):
                            psm = psum_m.tile([128, 512], f32, tag="psm")
                            for kd in range(KD):
                                nc.tensor.matmul(
                                    psm, tl[:, kd * 128:(kd + 1) * 128],
                                    embT[:, kd, n * 512:(n + 1) * 512],
                                    start=(kd == 0), stop=(kd == KD - 1))
                            expt = work.tile([128, 512], f32, tag="expt", bufs=exp_bufs)
                            nc.scalar.activation(out=expt, in_=psm, func=AF.Exp,
                                                 bias=biasM, scale=OIM_SCALAR)
                            scr = work.tile([128, 512], f32, tag="scr", bufs=exp_bufs)
                            nc.vector.scalar_tensor_tensor(
                                out=scr, in0=expt, scalar=1.0,
                                in1=keepg[:, n * 512:(n + 1) * 512],
                                op0=OP.mult, op1=OP.mult,
                                accum_out=acc4[:, n:n + 1])
                    nc.vector.reduce_sum(out=ssb[:, m:m + 1], in_=acc4,
                                         axis=mybir.AxisListType.X)

            nc.sync.dma_start(out=sume_d, in_=ssb)
            nc.sync.dma_start(out=tco_d, in_=tsb)
            tickt = singles.tile([128, 4], f32)
            nc.sync.dma_start(out=tickt, in_=tick_d)
            nc.sync.dma_start(out=tock_d, in_=tickt)

    nc.compile()
    return nc


def _host_bookkeeping(labels, label_cq, header_cq):
    """Mirror the reference's integer-only queue-update semantics."""
    labels = np.asarray(labels).astype(np.int64)
    lab = np.asarray(label_cq).astype(np.int64).copy()
    h0 = int(np.asarray(header_cq))

    # jnp.unique(labels, size=U): sorted unique, padded with the minimum
    uq = np.unique(labels)
    if uq.size < U:
        uniq = np.concatenate([uq, np.full(U - uq.size, uq.min(), np.int64)])
    else:
        uniq = uq[:U]
    cnts = np.array([(labels == v).sum() for v in uniq], np.int64)

    emb_src = np.full(Q, -1, np.int64)   # >=0: row u of uniq means; -1: original
    h = h0 % Q
    for u in range(U):
        y = uniq[u]
        m = lab == y
        i = int(np.argmax(m)) if m.any() else 0
        inval = bool(m.any()) and (i != h)
        emb_src[h] = u
        lab[h] = y
        if inval:
            lab[i] = IGNORE
        h = (h + 1) % Q

    good = lab != IGNORE
    goodidx = np.flatnonzero(good)
    gl = lab[goodidx]
    vals, first = np.unique(gl, return_index=True)
    pos = np.searchsorted(vals, labels)
    assert np.all(vals[np.clip(pos, 0, vals.size - 1)] == labels), \
        "batch label missing from queue"
    xe = goodidx[first[pos]]
    return uniq, cnts, emb_src, good, xe


def _prepare(inputs, labels, emb_cq, label_cq, header_cq):
    """Host bookkeeping -> (M, per-core input maps, extra-target indices, xe)."""
    inputs = np.ascontiguousarray(np.asarray(inputs, np.float32))
    emb_cq = np.ascontiguousarray(np.asarray(emb_cq, np.float32))

    uniq, cnts, emb_src, good, xe = _host_bookkeeping(labels, label_cq, header_cq)

    # safe upper bound for any logit: 30 * max row norm (uniq means have norm 1)
    max_nrm = float(np.sqrt((emb_cq.astype(np.float64) ** 2).sum(axis=1).max()))
    M = OIM_SCALAR * max(1.0, max_nrm) * 1.0000001

    w_idx = emb_src[xe].astype(np.float64)        # -1 for non-window targets
    extra = np.flatnonzero(w_idx < 0)             # handled on host (rare/none)

    def pmajor(v, cols):
        return np.ascontiguousarray(
            np.asarray(v, np.float32).reshape(cols, 128).T)

    base = {
        "x": inputs,
        "tick": np.zeros((128, 4), np.float32),
        "labf": pmajor(np.asarray(labels, np.float64), MT),
        "uniqf": np.ascontiguousarray(
            np.broadcast_to(uniq.astype(np.float32), (128, U))),
        "cnts": pmajor(cnts, UT),
        "widx": pmajor(w_idx, MT),
        "iota": np.ascontiguousarray(
            np.broadcast_to(np.arange(U, dtype=np.float32), (128, U))),
    }
    in_maps = []
    for c in range(N_CORES):
        sl = slice(c * QS, (c + 1) * QS)
        src = emb_src[sl]
        ohtT = np.zeros((U, QS), np.float32)
        j = np.flatnonzero(src >= 0)
        ohtT[src[j], j] = 1.0
        in_maps.append({
            **base,
            "emb": np.ascontiguousarray(emb_cq[sl]),
            "gkeep": np.ascontiguousarray(
                np.broadcast_to(good[sl].astype(np.float32), (128, QS))),
            "wkeep": pmajor((src < 0).astype(np.float32), QT),
            "oht": np.ascontiguousarray(
                ohtT.reshape(UT, 128, QS).transpose(1, 0, 2)),
        })
    return M, in_maps, extra, xe


def _combine(res_list, M, extra, xe, inputs, emb_cq):
    """Unshard / combine per-core partials into the scalar loss."""
    S = np.zeros(B, np.float64)
    for r in res_list:
        S += r["sume"].astype(np.float64).T.reshape(B)
    t_cos = res_list[0]["tco"].astype(np.float64).T.reshape(B)

    if extra.size:  # targets pointing at original (non-window) queue rows
        xb = np.asarray(inputs, np.float64)[extra]
        xb /= np.maximum(np.linalg.norm(xb, axis=1, keepdims=True), 1e-12)
        eb = np.asarray(emb_cq, np.float64)[xe[extra]]
        t_cos[extra] = (xb * eb).sum(axis=1)

    loss = np.mean(M + np.log(S) - OIM_SCALAR * t_cos)
    return np.array(loss, dtype=np.float32)


def kernel(inputs, labels, emb_cq, label_cq, age_cq, header_cq):
    from concourse.bass_utils import run_bass_kernel_spmd

    M, in_maps, extra, xe = _prepare(inputs, labels, emb_cq, label_cq, header_cq)

    key = round(M, 9)
    if key not in _PROG_CACHE:
        _PROG_CACHE[key] = _build_program(M)
    nc = _PROG_CACHE[key]

    res = run_bass_kernel_spmd(nc, in_maps, core_ids=list(range(N_CORES)))
    return _combine(res.results, M, extra, xe, inputs, emb_cq)



# revision 3
# speedup vs baseline: 1.4590x; 1.4590x over previous
"""OIM loss with circular queue — Trainium2 Bass kernel (8 NeuronCores).

Strategy
--------
The output is a scalar:  loss = mean_b [ logsumexp_{q in good}(30*cos(x_b, e_q))
                                         - 30*cos(x_b, e_{xe_b}) ]
where e is the circular queue after the (sequential, data-dependent) update.

Integer queue bookkeeping runs on the host.  The host also reshapes the
float inputs so the device never transposes or masks anything big:

  - the queue is rotated by `header` so the U-slot write window is always
    slots [0, U) of core 0; emb_cq arrives pre-transposed (d-major) in bf16
  - invalidated queue slots arrive as all-zero rows; the host subtracts
    their exact contribution (n_bad * exp(-M)) from the returned sums
  - x arrives both b-major (for the masked-mean matmul) and d-major
    (lhsT of the logits matmul), bf16
  - the label one-hot mask [B, U] arrives as a bf16 input

Device work per core (emb_cq tensor-parallel over Q, 2048 rows/core):
  - masked per-pid sums via mask^T @ x (one-hot matmul), normalized in
    place (normalizing the sum == normalizing the mean)
  - the normalized means are PE-transposed and blended into the first 256
    embT columns (window core only, via a per-core 0/1 scalar)
  - the big [128,QS] logits matmul in bf16; exp is ONE activation op per
    512-block: out = exp(scl_b * psum - M) with scl_b = 30/||x_b|| folding
    the input normalization, and accum_out producing the row-sum directly
  - the target cosine is gathered from the n=0 PSUM block with a one-hot
    DVE op (the window always holds every batch pid's embedding)

The host adds the 8 partial sums, fixes the zero-row correction, takes
log and means.
"""

import os
import sys

import numpy as np

for _p in ("/opt/trn_rl_repo", "/root/.axon_site/_ro/trn_rl_repo"):
    if os.path.isdir(_p) and _p not in sys.path:
        sys.path.insert(0, _p)

import ml_dtypes

BF16 = ml_dtypes.bfloat16

B, D, Q, U = 4096, 512, 16384, 256
N_CORES = 8
QS = Q // N_CORES          # queue rows per core
OIM_SCALAR = 30.0
IGNORE = -1
MT = B // 128              # 32 b-tiles
KD = D // 128              # 4 contraction chunks
NQ = QS // 512             # 4 matmul n-chunks per core
UT = U // 128              # 2 u-tiles

_PROG_CACHE = {}


def _build_program(M: float):
    """Emit + schedule + compile the (SPMD, identical on all cores) program."""
    import concourse.bacc as bacc
    import concourse.tile as tile
    from concourse import mybir
    from concourse.masks import make_identity

    f32 = mybir.dt.float32
    bf16 = mybir.dt.bfloat16
    AF = mybir.ActivationFunctionType
    OP = mybir.AluOpType

    nc = bacc.Bacc("TRN2", target_bir_lowering=False, debug=False,
                   num_devices=N_CORES)

    x_d = nc.dram_tensor("x", [B, D], bf16, kind="ExternalInput").ap()
    xt_d = nc.dram_tensor("xt", [128, KD, B], bf16, kind="ExternalInput").ap()
    embt_d = nc.dram_tensor("embt", [128, KD, QS], bf16,
                            kind="ExternalInput").ap()
    maskb_d = nc.dram_tensor("maskb", [128, MT, U], bf16,
                             kind="ExternalInput").ap()
    widx_d = nc.dram_tensor("widx", [128, MT], f32, kind="ExternalInput").ap()
    iota_d = nc.dram_tensor("iota", [128, U], f32, kind="ExternalInput").ap()
    wflag_d = nc.dram_tensor("wflag", [128, 1], f32, kind="ExternalInput").ap()
    sume_d = nc.dram_tensor("sume", [128, MT], f32, kind="ExternalOutput").ap()
    tco_d = nc.dram_tensor("tco", [128, MT], f32, kind="ExternalOutput").ap()
    tick_d = nc.dram_tensor("tick", [128, 4], f32, kind="ExternalInput").ap()
    tock_d = nc.dram_tensor("tock", [128, 4], f32, kind="ExternalOutput").ap()

    with tile.TileContext(nc) as tc:
        with (
            tc.tile_pool(name="singles", bufs=1) as singles,
            tc.tile_pool(name="work", bufs=4) as work,
            tc.tile_pool(name="small", bufs=4) as small,
            tc.tile_pool(name="psum_t", bufs=2, space="PSUM") as psum_t,
        ):
            # ---------------- resident tensors ----------------
            xT = singles.tile([128, KD, B], bf16)
            nc.sync.dma_start(out=xT, in_=xt_d)
            embT = singles.tile([128, KD, QS], bf16)
            nc.sync.dma_start(out=embT, in_=embt_d)
            maskb = singles.tile([128, MT, U], bf16)
            nc.sync.dma_start(out=maskb, in_=maskb_d)
            widx = singles.tile([128, MT], f32)
            nc.sync.dma_start(out=widx, in_=widx_d)
            iotab = singles.tile([128, U], f32)
            nc.sync.dma_start(out=iotab, in_=iota_d)
            wflag = singles.tile([128, 1], f32)
            nc.sync.dma_start(out=wflag, in_=wflag_d)

            identb = singles.tile([128, 128], bf16)
            make_identity(nc, identb)
            biasM = singles.tile([128, 1], f32)
            nc.vector.memset(biasM, -M)

            ssqc = singles.tile([128, MT], f32)   # sum x^2 per row
            scl = singles.tile([128, MT], f32)    # 30 / ||x_b||
            ssb = singles.tile([128, MT], f32)    # sum-exp out collector
            praw = singles.tile([128, MT], f32)   # raw target dot collector
            tsb = singles.tile([128, MT], f32)    # 30*cos(target) out
            uemb_bf = singles.tile([128, UT, D], bf16)

            # ------------- phase 1: masked sums + row norms -------------
            with tc.tile_pool(name="psum_u", bufs=1, space="PSUM") as psum_u:
                ps_u = [psum_u.tile([128, D], f32, tag=f"uniq{mu}",
                                    name=f"ps_u{mu}") for mu in range(UT)]
                for i in range(MT):
                    x_t = work.tile([128, D], bf16, tag="x")
                    nc.sync.dma_start(out=x_t, in_=x_d[i * 128:(i + 1) * 128, :])
                    for mu in range(UT):
                        nc.tensor.matmul(ps_u[mu],
                                         maskb[:, i, mu * 128:(mu + 1) * 128],
                                         x_t, start=(i == 0),
                                         stop=(i == MT - 1))
                    sq = work.tile([128, D], f32, tag="sq")
                    nc.vector.scalar_tensor_tensor(
                        out=sq, in0=x_t, scalar=1.0, in1=x_t,
                        op0=OP.mult, op1=OP.mult,
                        accum_out=ssqc[:, i:i + 1])

                # 30/||x_b||  (feeds the exp scale)
                nrm = small.tile([128, MT], f32, tag="nrm")
                nc.scalar.activation(out=nrm, in_=ssqc, func=AF.Sqrt)
                nc.vector.tensor_scalar_max(out=nrm, in0=nrm, scalar1=1e-12)
                rin = small.tile([128, MT], f32, tag="rin")
                nc.vector.reciprocal(rin, nrm)
                nc.vector.tensor_scalar_mul(out=scl, in0=rin,
                                            scalar1=OIM_SCALAR)

                # normalize the masked sums (== normalized means)
                ssqu = small.tile([128, UT], f32, tag="ssqu")
                for mu in range(UT):
                    squ = work.tile([128, D], f32, tag="sq")
                    nc.scalar.activation(out=squ, in_=ps_u[mu],
                                         func=AF.Square,
                                         accum_out=ssqu[:, mu:mu + 1])
                nrmu = small.tile([128, UT], f32, tag="nrmu")
                nc.scalar.activation(out=nrmu, in_=ssqu, func=AF.Sqrt)
                nc.vector.tensor_scalar_max(out=nrmu, in0=nrmu, scalar1=1e-12)
                rinu = small.tile([128, UT], f32, tag="rinu")
                nc.vector.reciprocal(rinu, nrmu)
                for mu in range(UT):
                    nc.vector.tensor_scalar_mul(out=uemb_bf[:, mu, :],
                                                in0=ps_u[mu],
                                                scalar1=rinu[:, mu:mu + 1])

                # transpose to d-major; blend into the embT window:
                # embT_win = embT_win + wflag * uembT   (wflag 1 on window core)
                for mu in range(UT):
                    for kd in range(KD):
                        pst = psum_t.tile([128, 128], bf16, tag="pst")
                        nc.tensor.transpose(
                            pst, uemb_bf[:, mu, kd * 128:(kd + 1) * 128],
                            identb)
                        nc.vector.scalar_tensor_tensor(
                            out=embT[:, kd, mu * 128:(mu + 1) * 128],
                            in0=pst, scalar=wflag,
                            in1=embT[:, kd, mu * 128:(mu + 1) * 128],
                            op0=OP.mult, op1=OP.add)

            # ------------- phase 2: logits + fused exp/sum -------------
            with tc.tile_pool(name="psum_m", bufs=4, space="PSUM") as psum_m:
                for m in range(MT):
                    acc4 = small.tile([128, NQ], f32, tag="acc4")
                    for n in range(NQ):
                        psm = psum_m.tile([128, 512], f32, tag="psm")
                        for kd in range(KD):
                            nc.tensor.matmul(
                                psm, xT[:, kd, m * 128:(m + 1) * 128],
                                embT[:, kd, n * 512:(n + 1) * 512],
                                start=(kd == 0), stop=(kd == KD - 1))
                        if n == 0:
                            # target dot: window col widx_b of this block
                            scr = work.tile([128, U], f32, tag="scr")
                            nc.vector.scalar_tensor_tensor(
                                out=scr, in0=iotab, scalar=widx[:, m:m + 1],
                                in1=psm[:, 0:U], op0=OP.is_equal, op1=OP.mult,
                                accum_out=praw[:, m:m + 1])
                        expt = work.tile([128, 512], f32, tag="expt", bufs=6)
                        nc.scalar.activation(out=expt, in_=psm, func=AF.Exp,
                                             bias=biasM, scale=scl[:, m:m + 1],
                                             accum_out=acc4[:, n:n + 1])
                    nc.vector.reduce_sum(out=ssb[:, m:m + 1], in_=acc4,
                                         axis=mybir.AxisListType.X)
                nc.vector.tensor_mul(out=tsb, in0=praw, in1=scl)

            nc.sync.dma_start(out=sume_d, in_=ssb)
            nc.sync.dma_start(out=tco_d, in_=tsb)
            tickt = singles.tile([128, 4], f32)
            nc.sync.dma_start(out=tickt, in_=tick_d)
            nc.sync.dma_start(out=tock_d, in_=tickt)

    nc.compile()
    return nc


def _host_bookkeeping(labels, label_cq, header_cq):
    """Mirror the reference's integer-only queue-update semantics."""
    labels = np.asarray(labels).astype(np.int64)
    lab = np.asarray(label_cq).astype(np.int64).copy()
    h0 = int(np.asarray(header_cq))

    # jnp.unique(labels, size=U): sorted unique, padded with the minimum
    uq = np.unique(labels)
    if uq.size < U:
        uniq = np.concatenate([uq, np.full(U - uq.size, uq.min(), np.int64)])
    else:
        uniq = uq[:U]

    emb_src = np.full(Q, -1, np.int64)   # >=0: row u of uniq means; -1: original
    h = h0 % Q
    for u in range(U):
        y = uniq[u]
        m = lab == y
        i = int(np.argmax(m)) if m.any() else 0
        inval = bool(m.any()) and (i != h)
        emb_src[h] = u
        lab[h] = y
        if inval:
            lab[i] = IGNORE
        h = (h + 1) % Q

    good = lab != IGNORE
    goodidx = np.flatnonzero(good)
    gl = lab[goodidx]
    vals, first = np.unique(gl, return_index=True)
    pos = np.searchsorted(vals, labels)
    assert np.all(vals[np.clip(pos, 0, vals.size - 1)] == labels), \
        "batch label missing from queue"
    xe = goodidx[first[pos]]
    return uniq, emb_src, good, xe, h0


def _prepare(inputs, labels, emb_cq, label_cq, header_cq):
    """Host bookkeeping -> (M, per-core input maps, extras, correction)."""
    inputs = np.ascontiguousarray(np.asarray(inputs, np.float32))
    emb_cq = np.ascontiguousarray(np.asarray(emb_cq, np.float32))

    uniq, emb_src, good, xe, h0 = _host_bookkeeping(labels, label_cq,
                                                    header_cq)

    # safe upper bound for any logit: 30 * max row norm (+bf16 slack)
    max_nrm = float(np.sqrt((emb_cq.astype(np.float64) ** 2).sum(axis=1).max()))
    M = OIM_SCALAR * max(1.0, max_nrm) * 1.01

    w_idx = emb_src[xe].astype(np.float64)        # -1 for non-window targets
    extra = np.flatnonzero(w_idx < 0)             # handled on host (rare/none)

    # rotate the queue so the window is slots [0, U) -> core 0, cols [0, U)
    rot = (h0 + np.arange(Q)) % Q
    emb_rot = emb_cq[rot].copy()
    good_rot = good[rot]
    src_rot = emb_src[rot]
    assert np.all(src_rot[:U] == np.arange(U)) and np.all(src_rot[U:] < 0)
    # zero all masked-out rows (stale slots) and the window rows (the device
    # adds the fresh means there); host subtracts the zero-row exp later
    zero_rows = ~good_rot
    zero_rows[:U] = True
    emb_rot[zero_rows] = 0.0
    n_bad = int((~good_rot[U:]).sum())            # zero rows that stay zero

    def dmajor(a):  # [R, D] f32 -> [128, KD, R] bf16
        r = a.shape[0]
        return np.ascontiguousarray(
            a.T.reshape(KD, 128, r).transpose(1, 0, 2).astype(BF16))

    lab2 = np.asarray(labels).reshape(MT, 128)
    maskb = (lab2[:, :, None] == uniq[None, None, :]).astype(BF16)
    maskb = np.ascontiguousarray(maskb.transpose(1, 0, 2))

    base = {
        "x": np.ascontiguousarray(inputs.astype(BF16)),
        "xt": dmajor(inputs),
        "maskb": maskb,
        "widx": np.ascontiguousarray(
            w_idx.reshape(MT, 128).T.astype(np.float32)),
        "iota": np.ascontiguousarray(
            np.broadcast_to(np.arange(U, dtype=np.float32), (128, U))),
        "tick": np.zeros((128, 4), np.float32),
    }
    in_maps = []
    for c in range(N_CORES):
        in_maps.append({
            **base,
            "embt": dmajor(emb_rot[c * QS:(c + 1) * QS]),
            "wflag": np.full((128, 1), 1.0 if c == 0 else 0.0, np.float32),
        })
    return M, in_maps, extra, xe, n_bad


def _combine(res_list, M, extra, xe, n_bad, inputs, emb_cq):
    """Unshard / combine per-core partials into the scalar loss."""
    S = np.zeros(B, np.float64)
    for r in res_list:
        S += r["sume"].astype(np.float64).T.reshape(B)
    S -= n_bad * np.exp(-float(M))                # zeroed rows' exp(0 - M)
    t30 = res_list[0]["tco"].astype(np.float64).T.reshape(B)

    if extra.size:  # targets pointing at original (non-window) queue rows
        xb = np.asarray(inputs, np.float64)[extra]
        xb /= np.maximum(np.linalg.norm(xb, axis=1, keepdims=True), 1e-12)
        eb = np.asarray(emb_cq, np.float64)[xe[extra]]
        t30[extra] = OIM_SCALAR * (xb * eb).sum(axis=1)

    loss = np.mean(M + np.log(S) - t30)
    return np.array(loss, dtype=np.float32)


def kernel(inputs, labels, emb_cq, label_cq, age_cq, header_cq):
    from concourse.bass_utils import run_bass_kernel_spmd

    M, in_maps, extra, xe, n_bad = _prepare(inputs, labels, emb_cq, label_cq,
                                            header_cq)

    key = round(M, 9)
    if key not in _PROG_CACHE:
        _PROG_CACHE[key] = _build_program(M)
    nc = _PROG_CACHE[key]

    res = run_bass_kernel_spmd(nc, in_maps, core_ids=list(range(N_CORES)))
    return _combine(res.results, M, extra, xe, n_bad, inputs, emb_cq)


# revision 5
# speedup vs baseline: 2.3603x; 1.6178x over previous
"""OIM loss with circular queue — Trainium2 Bass kernel (8 NeuronCores).

Strategy
--------
The output is a scalar:  loss = mean_b [ logsumexp_{q in good}(30*cos(x_b, e_q))
                                         - 30*cos(x_b, e_{xe_b}) ]
where e is the circular queue after the (sequential, data-dependent) update.

Integer queue bookkeeping and input reshaping run on the host; every
matmul/exp FLOP runs on the 8 cores:

  - the queue is rotated by `header` so the U-slot write window is always
    slots [0, U) of core 0; emb_cq arrives pre-transposed (d-major) in
    fp8e4m3, with invalidated slots zeroed (the host subtracts their exact
    n_bad * exp(-M) contribution from the returned sums)
  - x arrives b-major fp8 (masked-mean matmul) and d-major fp8 (logits
    lhsT), both in DoubleRow-paired layout; the label one-hot mask arrives
    fp8.  All matmuls use fp8 DoubleRow (contraction 256/pass)
  - per-row scale 30/||x_b|| comes precomputed from the f32 inputs and is
    applied inside the exp activation (per-partition scale), so normalized
    x never materializes
  - per core: masked-sum matmul -> normalize (Square/Sqrt on [128,2]) ->
    PE-transpose -> blend into the embT window (per-core 0/1 flag);
    the big matmul accumulates 4x512 blocks into one 4-bank PSUM tile and
    a SINGLE exp activation with accum_out produces each row-sum
  - the target cosine is gathered from cols [0,256) of the PSUM tile with
    a one-hot DVE op (the window holds every batch pid's embedding)

The host adds the 8 partial sums, fixes the zero-row correction, takes
log and means.
"""

import os
import sys

import numpy as np

for _p in ("/opt/trn_rl_repo", "/root/.axon_site/_ro/trn_rl_repo"):
    if os.path.isdir(_p) and _p not in sys.path:
        sys.path.insert(0, _p)

import ml_dtypes

BF16 = ml_dtypes.bfloat16
FP8 = ml_dtypes.float8_e4m3

B, D, Q, U = 4096, 512, 16384, 256
N_CORES = 8
QS = Q // N_CORES          # queue rows per core
OIM_SCALAR = 30.0
IGNORE = -1
MT = B // 128              # 32 b-tiles
MP = MT // 2               # 16 b-tile pairs (DoubleRow)
KD = D // 128              # 4 contraction chunks
KP = KD // 2               # 2 chunk pairs (DoubleRow)
NQ = QS // 512             # 4 matmul n-chunks per core
UT = U // 128              # 2 u-tiles

_PROG_CACHE = {}


def _build_program(M: float):
    """Emit + schedule + compile the (SPMD, identical on all cores) program."""
    import concourse.bacc as bacc
    import concourse.tile as tile
    from concourse import mybir
    from concourse.masks import make_identity

    f32 = mybir.dt.float32
    bf16 = mybir.dt.bfloat16
    fp8 = mybir.dt.float8e4
    AF = mybir.ActivationFunctionType
    OP = mybir.AluOpType
    DR = mybir.MatmulPerfMode.DoubleRow

    nc = bacc.Bacc("TRN2", target_bir_lowering=False, debug=False,
                   num_devices=N_CORES)

    x8_d = nc.dram_tensor("x8", [128, MP, 2, D], fp8, kind="ExternalInput").ap()
    xt_d = nc.dram_tensor("xt8", [128, KP, 2, B], fp8, kind="ExternalInput").ap()
    embt_d = nc.dram_tensor("embt8", [128, KP, 2, QS], fp8,
                            kind="ExternalInput").ap()
    mask_d = nc.dram_tensor("mask8", [128, MP, 2, U], fp8,
                            kind="ExternalInput").ap()
    scl_d = nc.dram_tensor("scl", [128, MT], f32, kind="ExternalInput").ap()
    widx_d = nc.dram_tensor("widx", [128, MT], f32, kind="ExternalInput").ap()
    iota_d = nc.dram_tensor("iota", [128, U], f32, kind="ExternalInput").ap()
    wflag_d = nc.dram_tensor("wflag", [128, 1], f32, kind="ExternalInput").ap()
    sume_d = nc.dram_tensor("sume", [128, MT], f32, kind="ExternalOutput").ap()
    tco_d = nc.dram_tensor("tco", [128, MT], f32, kind="ExternalOutput").ap()
    tick_d = nc.dram_tensor("tick", [128, 4], f32, kind="ExternalInput").ap()
    tock_d = nc.dram_tensor("tock", [128, 4], f32, kind="ExternalOutput").ap()

    with tile.TileContext(nc) as tc:
        with (
            tc.tile_pool(name="singles", bufs=1) as singles,
            tc.tile_pool(name="work", bufs=4) as work,
            tc.tile_pool(name="mpool", bufs=3) as mpool,
            tc.tile_pool(name="small", bufs=4) as small,
        ):
            # small consts first (cheap DMAs ahead of the big streams)
            scl = singles.tile([128, MT], f32)
            nc.sync.dma_start(out=scl, in_=scl_d)
            widx = singles.tile([128, MT], f32)
            nc.sync.dma_start(out=widx, in_=widx_d)
            iotab = singles.tile([128, U], f32)
            nc.sync.dma_start(out=iotab, in_=iota_d)
            wflag = singles.tile([128, 1], f32)
            nc.sync.dma_start(out=wflag, in_=wflag_d)
            tickt = singles.tile([128, 4], f32)
            nc.sync.dma_start(out=tickt, in_=tick_d)

            biasM = singles.tile([128, 1], f32)
            nc.vector.memset(biasM, -M)
            identb = singles.tile([128, 128], bf16)
            make_identity(nc, identb)

            # preload the ACT tables off the critical path (Exp set also
            # holds Square; Sqrt loads during the means stream)
            warm = small.tile([128, 1], f32, tag="warm")
            nc.vector.memset(warm, 1.0)
            warm2 = small.tile([128, 1], f32, tag="warm2")
            nc.scalar.activation(out=warm2, in_=warm, func=AF.Exp)

            ssb = singles.tile([128, MT], f32)    # sum-exp out collector
            praw = singles.tile([128, MT], f32)   # raw target dot collector
            tsb = singles.tile([128, MT], f32)    # 30*cos(target) out
            uemb_bf = singles.tile([128, UT, D], bf16)
            embT = singles.tile([128, KP, 2, QS], fp8)
            xT = singles.tile([128, KP, 2, B], fp8)

            # ------------- phase 1: masked sums (DoubleRow) -------------
            with (
                tc.tile_pool(name="psum_u", bufs=1, space="PSUM") as psum_u,
                tc.tile_pool(name="psum_t", bufs=2, space="PSUM") as psum_t,
            ):
                ps_u = [psum_u.tile([128, D], f32, tag=f"uniq{mu}",
                                    name=f"ps_u{mu}") for mu in range(UT)]
                for p in range(MP):
                    msk = mpool.tile([128, 2, U], fp8, tag="msk")
                    nc.sync.dma_start(out=msk, in_=mask_d[:, p, :, :])
                    x_t = work.tile([128, 2, D], fp8, tag="x")
                    nc.sync.dma_start(out=x_t, in_=x8_d[:, p, :, :])
                    for mu in range(UT):
                        nc.tensor.matmul(ps_u[mu],
                                         msk[:, :, mu * 128:(mu + 1) * 128],
                                         x_t, start=(p == 0),
                                         stop=(p == MP - 1), perf_mode=DR)

                # logits lhsT / rhs streams land right after the means stream
                nc.sync.dma_start(out=embT, in_=embt_d)
                nc.sync.dma_start(out=xT, in_=xt_d)

                # normalize the masked sums (== normalized means)
                ssqu = small.tile([128, UT], f32, tag="ssqu")
                for mu in range(UT):
                    squ = work.tile([128, D], f32, tag="squ")
                    nc.scalar.activation(out=squ, in_=ps_u[mu],
                                         func=AF.Square,
                                         accum_out=ssqu[:, mu:mu + 1])
                nrmu = small.tile([128, UT], f32, tag="nrmu")
                nc.scalar.activation(out=nrmu, in_=ssqu, func=AF.Sqrt)
                nc.vector.tensor_scalar_max(out=nrmu, in0=nrmu, scalar1=1e-12)
                rinu = small.tile([128, UT], f32, tag="rinu")
                nc.vector.reciprocal(rinu, nrmu)
                for mu in range(UT):
                    nc.vector.tensor_scalar_mul(out=uemb_bf[:, mu, :],
                                                in0=ps_u[mu],
                                                scalar1=rinu[:, mu:mu + 1])

                # transpose to d-major; blend into the embT window:
                # embT_win = embT_win + wflag * uembT   (wflag 1 on window core)
                for mu in range(UT):
                    for kd in range(KD):
                        pst = psum_t.tile([128, 128], bf16, tag="pst")
                        nc.tensor.transpose(
                            pst, uemb_bf[:, mu, kd * 128:(kd + 1) * 128],
                            identb)
                        nc.vector.scalar_tensor_tensor(
                            out=embT[:, kd // 2, kd % 2,
                                     mu * 128:(mu + 1) * 128],
                            in0=pst, scalar=wflag,
                            in1=embT[:, kd // 2, kd % 2,
                                     mu * 128:(mu + 1) * 128],
                            op0=OP.mult, op1=OP.add)

            # ------------- phase 2: logits + fused exp/sum -------------
            with tc.tile_pool(name="psum_m", bufs=2, space="PSUM") as psum_m:
                for m in range(MT):
                    psm = psum_m.tile([128, NQ * 512], f32, tag="psm")
                    for n in range(NQ):
                        for j in range(KP):
                            nc.tensor.matmul(
                                psm[:, n * 512:(n + 1) * 512],
                                xT[:, j, :, m * 128:(m + 1) * 128],
                                embT[:, j, :, n * 512:(n + 1) * 512],
                                start=(j == 0), stop=(j == KP - 1),
                                perf_mode=DR)
                    # target dot: window col widx_b of block 0
                    scr = work.tile([128, U], f32, tag="scr")
                    nc.vector.scalar_tensor_tensor(
                        out=scr, in0=iotab, scalar=widx[:, m:m + 1],
                        in1=psm[:, 0:U], op0=OP.is_equal, op1=OP.mult,
                        accum_out=praw[:, m:m + 1])
                    expt = work.tile([128, NQ * 512], f32, tag="expt", bufs=3)
                    nc.scalar.activation(out=expt, in_=psm, func=AF.Exp,
                                         bias=biasM, scale=scl[:, m:m + 1],
                                         accum_out=ssb[:, m:m + 1])
                nc.vector.tensor_mul(out=tsb, in0=praw, in1=scl)

            nc.sync.dma_start(out=sume_d, in_=ssb)
            nc.sync.dma_start(out=tco_d, in_=tsb)
            nc.sync.dma_start(out=tock_d, in_=tickt)

    nc.compile()
    return nc


def _host_bookkeeping(labels, label_cq, header_cq):
    """Mirror the reference's integer-only queue-update semantics."""
    labels = np.asarray(labels).astype(np.int64)
    lab = np.asarray(label_cq).astype(np.int64).copy()
    h0 = int(np.asarray(header_cq))

    # jnp.unique(labels, size=U): sorted unique, padded with the minimum
    uq = np.unique(labels)
    if uq.size < U:
        uniq = np.concatenate([uq, np.full(U - uq.size, uq.min(), np.int64)])
    else:
        uniq = uq[:U]

    emb_src = np.full(Q, -1, np.int64)   # >=0: row u of uniq means; -1: original
    h = h0 % Q
    for u in range(U):
        y = uniq[u]
        m = lab == y
        i = int(np.argmax(m)) if m.any() else 0
        inval = bool(m.any()) and (i != h)
        emb_src[h] = u
        lab[h] = y
        if inval:
            lab[i] = IGNORE
        h = (h + 1) % Q

    good = lab != IGNORE
    goodidx = np.flatnonzero(good)
    gl = lab[goodidx]
    vals, first = np.unique(gl, return_index=True)
    pos = np.searchsorted(vals, labels)
    assert np.all(vals[np.clip(pos, 0, vals.size - 1)] == labels), \
        "batch label missing from queue"
    xe = goodidx[first[pos]]
    return uniq, emb_src, good, xe, h0


def _prepare(inputs, labels, emb_cq, label_cq, header_cq):
    """Host bookkeeping -> (M, per-core input maps, extras, correction)."""
    inputs = np.ascontiguousarray(np.asarray(inputs, np.float32))
    emb_cq = np.ascontiguousarray(np.asarray(emb_cq, np.float32))

    uniq, emb_src, good, xe, h0 = _host_bookkeeping(labels, label_cq,
                                                    header_cq)

    # safe upper bound for any logit: 30 * max row norm (+fp8 slack)
    max_nrm = float(np.sqrt((emb_cq.astype(np.float64) ** 2).sum(axis=1).max()))
    M = OIM_SCALAR * max(1.0, max_nrm) * 1.10

    w_idx = emb_src[xe].astype(np.float64)        # -1 for non-window targets
    extra = np.flatnonzero(w_idx < 0)             # handled on host (rare/none)

    # rotate the queue so the window is slots [0, U) -> core 0, cols [0, U)
    rot = (h0 + np.arange(Q)) % Q
    emb_rot = emb_cq[rot].copy()
    good_rot = good[rot]
    src_rot = emb_src[rot]
    assert np.all(src_rot[:U] == np.arange(U)) and np.all(src_rot[U:] < 0)
    # zero all masked-out rows (stale slots) and the window rows (the device
    # adds the fresh means there); host subtracts the zero-row exp later
    zero_rows = ~good_rot
    zero_rows[:U] = True
    emb_rot[zero_rows] = 0.0
    n_bad = int((~good_rot[U:]).sum())            # zero rows that stay zero

    def dmajor8(a):  # [R, D] f32 -> [128, KP, 2, R] fp8 (DoubleRow pairs)
        r = a.shape[0]
        return np.ascontiguousarray(
            a.T.reshape(KP, 2, 128, r).transpose(2, 0, 1, 3).astype(FP8))

    # 30 / ||x_b|| in the device's [128, MT] layout (p-major)
    nrm = np.sqrt((inputs.astype(np.float64) ** 2).sum(axis=1))
    sclv = OIM_SCALAR / np.maximum(nrm, 1e-12)

    lab2 = np.asarray(labels).reshape(MP, 2, 128)
    uniqv = uniq
    mask8 = (lab2[:, :, :, None] == uniqv[None, None, None, :]).astype(FP8)
    mask8 = np.ascontiguousarray(mask8.transpose(2, 0, 1, 3))

    x8 = np.ascontiguousarray(
        inputs.reshape(MP, 2, 128, D).transpose(2, 0, 1, 3).astype(FP8))

    base = {
        "x8": x8,
        "xt8": dmajor8(inputs),
        "mask8": mask8,
        "scl": np.ascontiguousarray(
            sclv.reshape(MT, 128).T.astype(np.float32)),
        "widx": np.ascontiguousarray(
            w_idx.reshape(MT, 128).T.astype(np.float32)),
        "iota": np.ascontiguousarray(
            np.broadcast_to(np.arange(U, dtype=np.float32), (128, U))),
        "tick": np.zeros((128, 4), np.float32),
    }
    in_maps = []
    for c in range(N_CORES):
        in_maps.append({
            **base,
            "embt8": dmajor8(emb_rot[c * QS:(c + 1) * QS]),
            "wflag": np.full((128, 1), 1.0 if c == 0 else 0.0, np.float32),
        })
    return M, in_maps, extra, xe, n_bad


def _combine(res_list, M, extra, xe, n_bad, inputs, emb_cq):
    """Unshard / combine per-core partials into the scalar loss."""
    S = np.zeros(B, np.float64)
    for r in res_list:
        S += r["sume"].astype(np.float64).T.reshape(B)
    S -= n_bad * np.exp(-float(M))                # zeroed rows' exp(0 - M)
    t30 = res_list[0]["tco"].astype(np.float64).T.reshape(B)

    if extra.size:  # targets pointing at original (non-window) queue rows
        xb = np.asarray(inputs, np.float64)[extra]
        xb /= np.maximum(np.linalg.norm(xb, axis=1, keepdims=True), 1e-12)
        eb = np.asarray(emb_cq, np.float64)[xe[extra]]
        t30[extra] = OIM_SCALAR * (xb * eb).sum(axis=1)

    loss = np.mean(M + np.log(S) - t30)
    return np.array(loss, dtype=np.float32)


def kernel(inputs, labels, emb_cq, label_cq, age_cq, header_cq):
    from concourse.bass_utils import run_bass_kernel_spmd

    M, in_maps, extra, xe, n_bad = _prepare(inputs, labels, emb_cq, label_cq,
                                            header_cq)

    key = round(M, 9)
    if key not in _PROG_CACHE:
        _PROG_CACHE[key] = _build_program(M)
    nc = _PROG_CACHE[key]

    res = run_bass_kernel_spmd(nc, in_maps, core_ids=list(range(N_CORES)))
    return _combine(res.results, M, extra, xe, n_bad, inputs, emb_cq)


# revision 9
# speedup vs baseline: 2.5196x; 1.0675x over previous
"""OIM loss with circular queue — Trainium2 Bass kernel (8 NeuronCores).

Strategy
--------
The output is a scalar:  loss = mean_b [ logsumexp_{q in good}(30*cos(x_b, e_q))
                                         - 30*cos(x_b, e_{xe_b}) ]
where e is the circular queue after the (sequential, data-dependent) update.

Integer queue bookkeeping and input reshaping run on the host; every
matmul/exp FLOP runs on the 8 cores:

  - the queue is rotated by `header` so the U-slot write window is always
    slots [0, U) of core 0; emb_cq arrives pre-transposed (d-major) in
    fp8e4m3, with invalidated slots zeroed (the host subtracts their exact
    n_bad * exp(-M) contribution from the returned sums)
  - x arrives b-major fp8 (masked-mean matmul) and d-major fp8 (logits
    lhsT), both in DoubleRow-paired layout; the label one-hot mask arrives
    fp8.  All matmuls use fp8 DoubleRow (contraction 256/pass)
  - per-row scale 30/||x_b|| comes precomputed from the f32 inputs and is
    applied inside the exp activation (per-partition scale), so normalized
    x never materializes
  - per core: masked-sum matmul -> normalize (Square/Sqrt on [128,2]) ->
    PE-transpose -> blend into the embT window (per-core 0/1 flag);
    the big matmul accumulates 4x512 blocks into one 4-bank PSUM tile and
    a SINGLE exp activation with accum_out produces each row-sum
  - the target cosine is gathered from cols [0,256) of the PSUM tile with
    a one-hot DVE op (the window holds every batch pid's embedding)

The host adds the 8 partial sums, fixes the zero-row correction, takes
log and means.
"""

import os
import sys

import numpy as np

for _p in ("/opt/trn_rl_repo", "/root/.axon_site/_ro/trn_rl_repo"):
    if os.path.isdir(_p) and _p not in sys.path:
        sys.path.insert(0, _p)

import ml_dtypes

BF16 = ml_dtypes.bfloat16
FP8 = ml_dtypes.float8_e4m3

B, D, Q, U = 4096, 512, 16384, 256
N_CORES = 8
QS = Q // N_CORES          # queue rows per core
OIM_SCALAR = 30.0
IGNORE = -1
MT = B // 128              # 32 b-tiles
MP = MT // 2               # 16 b-tile pairs (DoubleRow)
KD = D // 128              # 4 contraction chunks
KP = KD // 2               # 2 chunk pairs (DoubleRow)
NQ = QS // 512             # 4 matmul n-chunks per core
UT = U // 128              # 2 u-tiles

_PROG_CACHE = {}


def _build_program(M: float):
    """Emit + schedule + compile the (SPMD, identical on all cores) program."""
    import concourse.bacc as bacc
    import concourse.tile as tile
    from concourse import mybir
    from concourse.masks import make_identity

    f32 = mybir.dt.float32
    bf16 = mybir.dt.bfloat16
    fp8 = mybir.dt.float8e4
    AF = mybir.ActivationFunctionType
    OP = mybir.AluOpType
    DR = mybir.MatmulPerfMode.DoubleRow

    nc = bacc.Bacc("TRN2", target_bir_lowering=False, debug=False,
                   num_devices=N_CORES)

    # consts columns: scl | widx | iota | wflag | tick
    CW = MT + MT + U + 1 + 4
    x8_d = nc.dram_tensor("x8", [128, MP, 2, D], fp8, kind="ExternalInput").ap()
    xt_d = nc.dram_tensor("xt8", [128, KP, 2, B], fp8, kind="ExternalInput").ap()
    embt_d = nc.dram_tensor("embt8", [128, KP, 2, QS], fp8,
                            kind="ExternalInput").ap()
    mask_d = nc.dram_tensor("mask8", [128, MP, 2, U], fp8,
                            kind="ExternalInput").ap()
    consts_d = nc.dram_tensor("consts", [128, CW], f32,
                              kind="ExternalInput").ap()
    sume_d = nc.dram_tensor("sume", [128, MT], f32, kind="ExternalOutput").ap()
    tco_d = nc.dram_tensor("tco", [128, MT], f32, kind="ExternalOutput").ap()
    tock_d = nc.dram_tensor("tock", [128, 4], f32, kind="ExternalOutput").ap()

    with tile.TileContext(nc) as tc:
        with (
            tc.tile_pool(name="singles", bufs=1) as singles,
            tc.tile_pool(name="work", bufs=4) as work,
            tc.tile_pool(name="mpool", bufs=3) as mpool,
            tc.tile_pool(name="small", bufs=4) as small,
        ):
            # one merged consts DMA:  scl | widx | iota | wflag | tick
            consts = singles.tile([128, CW], f32)
            nc.sync.dma_start(out=consts, in_=consts_d)
            scl = consts[:, 0:MT]
            widx = consts[:, MT:2 * MT]
            iotab = consts[:, 2 * MT:2 * MT + U]
            wflag = consts[:, 2 * MT + U:2 * MT + U + 1]
            tickt = consts[:, 2 * MT + U + 1:CW]

            biasM = singles.tile([128, 1], f32)
            nc.vector.memset(biasM, -M)
            identb = singles.tile([128, 128], bf16)
            make_identity(nc, identb)

            # preload the ACT tables off the critical path: Ln then Exp
            # (with Ln in the mix the shared natural_log_exp set serves
            # Ln/Exp/Square for the whole program)
            warm = small.tile([128, 1], f32, tag="warm")
            nc.vector.memset(warm, 1.0)
            warm2 = small.tile([128, 1], f32, tag="warm2")
            nc.scalar.activation(out=warm2, in_=warm, func=AF.Ln)
            nc.scalar.activation(out=warm2, in_=warm, func=AF.Exp)

            ssb = singles.tile([128, MT], f32)    # sum-exp out collector
            praw = singles.tile([128, MT], f32)   # raw target dot collector
            tsb = singles.tile([128, MT], f32)    # 30*cos(target) out
            uemb_bf = singles.tile([128, UT, D], bf16)
            mask8 = singles.tile([128, MP, 2, U], fp8)
            x8 = singles.tile([128, MP, 2, D], fp8)
            embT = singles.tile([128, KP, 2, QS], fp8)
            xT = singles.tile([128, KP, 2, B], fp8)

            # big input streams, in consumption order; each is one DMA so
            # the HWDGE descriptor cost (~0.6us each) stays off the path
            nc.sync.dma_start(out=mask8, in_=mask_d)
            nc.sync.dma_start(out=x8, in_=x8_d)
            nc.sync.dma_start(out=embT, in_=embt_d)
            nc.sync.dma_start(out=xT[:, :, :, 0:B // 2],
                              in_=xt_d[:, :, :, 0:B // 2])
            nc.sync.dma_start(out=xT[:, :, :, B // 2:B],
                              in_=xt_d[:, :, :, B // 2:B])

            # ------------- phase 1: masked sums (DoubleRow) -------------
            with (
                tc.tile_pool(name="psum_u", bufs=1, space="PSUM") as psum_u,
                tc.tile_pool(name="psum_t", bufs=2, space="PSUM") as psum_t,
            ):
                ps_u = [psum_u.tile([128, D], f32, tag=f"uniq{mu}",
                                    name=f"ps_u{mu}") for mu in range(UT)]
                for p in range(MP):
                    for mu in range(UT):
                        nc.tensor.matmul(ps_u[mu],
                                         mask8[:, p, :, mu * 128:(mu + 1) * 128],
                                         x8[:, p, :, :], start=(p == 0),
                                         stop=(p == MP - 1), perf_mode=DR)

                # normalize the masked sums (== normalized means);
                # 1/sqrt(s) = exp(-0.5*ln(s)) keeps everything on one table
                ssqu = small.tile([128, UT], f32, tag="ssqu")
                for mu in range(UT):
                    squ = work.tile([128, D], f32, tag="squ")
                    nc.scalar.activation(out=squ, in_=ps_u[mu],
                                         func=AF.Square,
                                         accum_out=ssqu[:, mu:mu + 1])
                nc.vector.tensor_scalar_max(out=ssqu, in0=ssqu, scalar1=1e-24)
                lnu = small.tile([128, UT], f32, tag="lnu")
                nc.scalar.activation(out=lnu, in_=ssqu, func=AF.Ln)
                rinu = small.tile([128, UT], f32, tag="rinu")
                nc.scalar.activation(out=rinu, in_=lnu, func=AF.Exp,
                                     scale=-0.5)
                for mu in range(UT):
                    nc.vector.tensor_scalar_mul(out=uemb_bf[:, mu, :],
                                                in0=ps_u[mu],
                                                scalar1=rinu[:, mu:mu + 1])

                # transpose to d-major; blend into the embT window:
                # embT_win = embT_win + wflag * uembT   (wflag 1 on window core)
                for mu in range(UT):
                    for kd in range(KD):
                        pst = psum_t.tile([128, 128], bf16, tag="pst")
                        nc.tensor.transpose(
                            pst, uemb_bf[:, mu, kd * 128:(kd + 1) * 128],
                            identb)
                        nc.vector.scalar_tensor_tensor(
                            out=embT[:, kd // 2, kd % 2,
                                     mu * 128:(mu + 1) * 128],
                            in0=pst, scalar=wflag,
                            in1=embT[:, kd // 2, kd % 2,
                                     mu * 128:(mu + 1) * 128],
                            op0=OP.mult, op1=OP.add)

            # ------------- phase 2: logits + fused exp/sum -------------
            with tc.tile_pool(name="psum_m", bufs=2, space="PSUM") as psum_m:
                for m in range(MT):
                    psm = psum_m.tile([128, NQ * 512], f32, tag="psm")
                    for n in range(NQ - 1, -1, -1):   # window block (n=0) last
                        for j in range(KP):
                            nc.tensor.matmul(
                                psm[:, n * 512:(n + 1) * 512],
                                xT[:, j, :, m * 128:(m + 1) * 128],
                                embT[:, j, :, n * 512:(n + 1) * 512],
                                start=(j == 0), stop=(j == KP - 1),
                                perf_mode=DR)
                    # target dot: window col widx_b of block 0
                    scr = work.tile([128, U], f32, tag="scr")
                    nc.vector.scalar_tensor_tensor(
                        out=scr, in0=iotab, scalar=widx[:, m:m + 1],
                        in1=psm[:, 0:U], op0=OP.is_equal, op1=OP.mult,
                        accum_out=praw[:, m:m + 1])
                    expt = work.tile([128, NQ * 512], f32, tag="expt", bufs=3)
                    nc.scalar.activation(out=expt, in_=psm, func=AF.Exp,
                                         bias=biasM, scale=scl[:, m:m + 1],
                                         accum_out=ssb[:, m:m + 1])
                nc.vector.tensor_mul(out=tsb, in0=praw, in1=scl)

            nc.sync.dma_start(out=sume_d, in_=ssb)
            nc.sync.dma_start(out=tco_d, in_=tsb)
            nc.sync.dma_start(out=tock_d, in_=tickt)

    nc.compile()
    return nc


def _host_bookkeeping(labels, label_cq, header_cq):
    """Mirror the reference's integer-only queue-update semantics."""
    labels = np.asarray(labels).astype(np.int64)
    lab = np.asarray(label_cq).astype(np.int64).copy()
    h0 = int(np.asarray(header_cq))

    # jnp.unique(labels, size=U): sorted unique, padded with the minimum
    uq = np.unique(labels)
    if uq.size < U:
        uniq = np.concatenate([uq, np.full(U - uq.size, uq.min(), np.int64)])
    else:
        uniq = uq[:U]

    emb_src = np.full(Q, -1, np.int64)   # >=0: row u of uniq means; -1: original
    h = h0 % Q
    for u in range(U):
        y = uniq[u]
        m = lab == y
        i = int(np.argmax(m)) if m.any() else 0
        inval = bool(m.any()) and (i != h)
        emb_src[h] = u
        lab[h] = y
        if inval:
            lab[i] = IGNORE
        h = (h + 1) % Q

    good = lab != IGNORE
    goodidx = np.flatnonzero(good)
    gl = lab[goodidx]
    vals, first = np.unique(gl, return_index=True)
    pos = np.searchsorted(vals, labels)
    assert np.all(vals[np.clip(pos, 0, vals.size - 1)] == labels), \
        "batch label missing from queue"
    xe = goodidx[first[pos]]
    return uniq, emb_src, good, xe, h0


def _prepare(inputs, labels, emb_cq, label_cq, header_cq):
    """Host bookkeeping -> (M, per-core input maps, extras, correction)."""
    inputs = np.ascontiguousarray(np.asarray(inputs, np.float32))
    emb_cq = np.ascontiguousarray(np.asarray(emb_cq, np.float32))

    uniq, emb_src, good, xe, h0 = _host_bookkeeping(labels, label_cq,
                                                    header_cq)

    # safe upper bound for any logit: 30 * max row norm (+fp8 slack)
    max_nrm = float(np.sqrt((emb_cq.astype(np.float64) ** 2).sum(axis=1).max()))
    M = OIM_SCALAR * max(1.0, max_nrm) * 1.10

    w_idx = emb_src[xe].astype(np.float64)        # -1 for non-window targets
    extra = np.flatnonzero(w_idx < 0)             # handled on host (rare/none)

    # rotate the queue so the window is slots [0, U) -> core 0, cols [0, U)
    rot = (h0 + np.arange(Q)) % Q
    emb_rot = emb_cq[rot].copy()
    good_rot = good[rot]
    src_rot = emb_src[rot]
    assert np.all(src_rot[:U] == np.arange(U)) and np.all(src_rot[U:] < 0)
    # zero all masked-out rows (stale slots) and the window rows (the device
    # adds the fresh means there); host subtracts the zero-row exp later
    zero_rows = ~good_rot
    zero_rows[:U] = True
    emb_rot[zero_rows] = 0.0
    n_bad = int((~good_rot[U:]).sum())            # zero rows that stay zero

    def dmajor8(a):  # [R, D] f32 -> [128, KP, 2, R] fp8 (DoubleRow pairs)
        r = a.shape[0]
        return np.ascontiguousarray(
            a.T.reshape(KP, 2, 128, r).transpose(2, 0, 1, 3).astype(FP8))

    # 30 / ||x_b|| in the device's [128, MT] layout (p-major)
    nrm = np.sqrt((inputs.astype(np.float64) ** 2).sum(axis=1))
    sclv = OIM_SCALAR / np.maximum(nrm, 1e-12)

    lab2 = np.asarray(labels).reshape(MP, 2, 128)
    uniqv = uniq
    mask8 = (lab2[:, :, :, None] == uniqv[None, None, None, :]).astype(FP8)
    mask8 = np.ascontiguousarray(mask8.transpose(2, 0, 1, 3))

    x8 = np.ascontiguousarray(
        inputs.reshape(MP, 2, 128, D).transpose(2, 0, 1, 3).astype(FP8))

    base = {
        "x8": x8,
        "xt8": dmajor8(inputs),
        "mask8": mask8,
    }
    in_maps = []
    for c in range(N_CORES):
        # consts columns: scl | widx | iota | wflag | tick
        consts = np.concatenate([
            sclv.reshape(MT, 128).T.astype(np.float32),
            w_idx.reshape(MT, 128).T.astype(np.float32),
            np.broadcast_to(np.arange(U, dtype=np.float32), (128, U)),
            np.full((128, 1), 1.0 if c == 0 else 0.0, np.float32),
            np.zeros((128, 4), np.float32),
        ], axis=1)
        in_maps.append({
            **base,
            "consts": np.ascontiguousarray(consts),
            "embt8": dmajor8(emb_rot[c * QS:(c + 1) * QS]),
        })
    return M, in_maps, extra, xe, n_bad


def _combine(res_list, M, extra, xe, n_bad, inputs, emb_cq):
    """Unshard / combine per-core partials into the scalar loss."""
    S = np.zeros(B, np.float64)
    for r in res_list:
        S += r["sume"].astype(np.float64).T.reshape(B)
    S -= n_bad * np.exp(-float(M))                # zeroed rows' exp(0 - M)
    t30 = res_list[0]["tco"].astype(np.float64).T.reshape(B)

    if extra.size:  # targets pointing at original (non-window) queue rows
        xb = np.asarray(inputs, np.float64)[extra]
        xb /= np.maximum(np.linalg.norm(xb, axis=1, keepdims=True), 1e-12)
        eb = np.asarray(emb_cq, np.float64)[xe[extra]]
        t30[extra] = OIM_SCALAR * (xb * eb).sum(axis=1)

    loss = np.mean(M + np.log(S) - t30)
    return np.array(loss, dtype=np.float32)


def kernel(inputs, labels, emb_cq, label_cq, age_cq, header_cq):
    from concourse.bass_utils import run_bass_kernel_spmd

    M, in_maps, extra, xe, n_bad = _prepare(inputs, labels, emb_cq, label_cq,
                                            header_cq)

    key = round(M, 9)
    if key not in _PROG_CACHE:
        _PROG_CACHE[key] = _build_program(M)
    nc = _PROG_CACHE[key]

    res = run_bass_kernel_spmd(nc, in_maps, core_ids=list(range(N_CORES)))
    return _combine(res.results, M, extra, xe, n_bad, inputs, emb_cq)


# revision 22
# speedup vs baseline: 3.5474x; 1.4079x over previous
"""OIM loss with circular queue — Trainium2 Bass kernel (8 NeuronCores).

Strategy
--------
The output is a scalar:  loss = mean_b [ logsumexp_{q in good}(30*cos(x_b, e_q))
                                         - 30*cos(x_b, e_{xe_b}) ]
where e is the circular queue after the (sequential, data-dependent) update.

All O(B*D + Q*D) bookkeeping and reshaping runs on the host: the integer
queue update, the per-pid masked means + queue-window write (4 MFLOP), the
rotation of the queue so the window is always core 0 / cols [0,U), zeroing
of invalidated slots (their exact n_bad * exp(-M) contribution is
subtracted from the device sums), fp8 quantization, and the d-major
DoubleRow layouts.  The O(B*Q*D) = 68.7 GFLOP logits matmul and the
B*Q = 67M-element exp/log-sum-exp run on the 8 cores, tensor-parallel
over Q (2048 queue rows per core):

  per m-tile of 128 rows: 8 fp8 DoubleRow matmuls accumulate the [128,2048]
  logits into one 4-bank PSUM tile; ONE exp activation per tile applies the
  per-row scale 30/||x_b|| (input normalization folded into the activation
  scale) and bias -M; row-sums of the bf16 exp output run on DVE/GPSIMD
  (alternating, to stay off the ACT critical path); the target cosine is
  gathered from cols [0,256) of PSUM with a one-hot DVE op (the window
  holds every batch pid's embedding).

The host adds the 8 partial sums, applies the zero-row correction, takes
log and means.
"""

import os
import sys

import numpy as np

for _p in ("/opt/trn_rl_repo", "/root/.axon_site/_ro/trn_rl_repo"):
    if os.path.isdir(_p) and _p not in sys.path:
        sys.path.insert(0, _p)

import ml_dtypes

BF16 = ml_dtypes.bfloat16
FP8 = ml_dtypes.float8_e4m3

B, D, Q, U = 4096, 512, 16384, 256
N_CORES = 8
QS = Q // N_CORES          # queue rows per core
OIM_SCALAR = 30.0
IGNORE = -1
MT = B // 128              # 32 b-tiles
KD = D // 128              # 4 contraction chunks
KP = KD // 2               # 2 chunk pairs (DoubleRow)
NQ = QS // 512             # 4 matmul n-chunks per core

_PROG_CACHE = {}


def _build_program(M: float):
    """Emit + schedule + compile the (SPMD, identical on all cores) program."""
    import concourse.bacc as bacc
    import concourse.tile as tile
    from concourse import mybir
    from concourse.masks import make_identity

    f32 = mybir.dt.float32
    bf16 = mybir.dt.bfloat16
    fp8 = mybir.dt.float8e4
    AF = mybir.ActivationFunctionType
    OP = mybir.AluOpType
    DR = mybir.MatmulPerfMode.DoubleRow

    nc = bacc.Bacc("TRN2", target_bir_lowering=False, debug=False,
                   num_devices=N_CORES)

    # consts columns: scl | widx | iota | tick
    CW = MT + MT + U + 4
    xt_d = nc.dram_tensor("xt8", [128, KP, 2, B], fp8, kind="ExternalInput").ap()
    embt_d = nc.dram_tensor("embt8", [128, KP, 2, QS], fp8,
                            kind="ExternalInput").ap()
    consts_d = nc.dram_tensor("consts", [128, CW], f32,
                              kind="ExternalInput").ap()
    sume_d = nc.dram_tensor("sume", [128, MT], f32, kind="ExternalOutput").ap()
    tco_d = nc.dram_tensor("tco", [128, MT], f32, kind="ExternalOutput").ap()
    tock_d = nc.dram_tensor("tock", [128, 4], f32, kind="ExternalOutput").ap()

    with tile.TileContext(nc) as tc:
        with (
            tc.tile_pool(name="singles", bufs=1) as singles,
            tc.tile_pool(name="work", bufs=4) as work,
            tc.tile_pool(name="small", bufs=4) as small,
        ):
            # one merged consts DMA:  scl | widx | iota | tick
            consts = singles.tile([128, CW], f32)
            nc.sync.dma_start(out=consts, in_=consts_d)
            scl = consts[:, 0:MT]
            widx = consts[:, MT:2 * MT]
            iotab = consts[:, 2 * MT:2 * MT + U]
            tickt = consts[:, 2 * MT + U:CW]

            biasM = singles.tile([128, 1], f32)
            nc.vector.memset(biasM, -M)
            identb = singles.tile([128, 128], bf16)
            make_identity(nc, identb)

            # preload the Exp table off the critical path (the only ACT func)
            warm = small.tile([128, 1], f32, tag="warm")
            nc.vector.memset(warm, 1.0)
            warm2 = small.tile([128, 1], f32, tag="warm2")
            nc.scalar.activation(out=warm2, in_=warm, func=AF.Exp)

            ssb = singles.tile([128, MT], f32)    # sum-exp out collector
            praw = singles.tile([128, MT], f32)   # exp(target logit - M) out
            embT = singles.tile([128, KP, 2, QS], fp8)
            xT = singles.tile([128, KP, 2, B], fp8)

            # tick/tock passthrough early (no compute dependency)
            nc.sync.dma_start(out=tock_d, in_=tickt)

            # input streams in consumption order (n runs 3..0, m runs 0..31):
            # fine chunks up front so the first PSUM tile completes early,
            # coarse chunks after
            nc.sync.dma_start(out=embT[:, :, :, 3 * QS // 4:QS],
                              in_=embt_d[:, :, :, 3 * QS // 4:QS])
            nc.sync.dma_start(out=xT[:, :, :, 0:B // 8],
                              in_=xt_d[:, :, :, 0:B // 8])
            for c in (2, 1, 0):
                nc.sync.dma_start(
                    out=embT[:, :, :, c * QS // 4:(c + 1) * QS // 4],
                    in_=embt_d[:, :, :, c * QS // 4:(c + 1) * QS // 4])
            for c in range(1, 8):
                nc.sync.dma_start(out=xT[:, :, :, c * B // 8:(c + 1) * B // 8],
                                  in_=xt_d[:, :, :, c * B // 8:(c + 1) * B // 8])

            # spin the PE p-state up with throwaway transposes so the first
            # logits matmuls run at full clock the moment their DMAs land
            with tc.tile_pool(name="psum_w", bufs=1, space="PSUM") as psum_w:
                wps = psum_w.tile([128, 128], bf16, tag="wps")
                for _ in range(34):
                    nc.tensor.transpose(wps, identb, identb)

            # ---------------- logits + exp + row sums ----------------
            with tc.tile_pool(name="psum_m", bufs=2, space="PSUM") as psum_m:
                for m in range(MT):
                    psm = psum_m.tile([128, NQ * 512], f32, tag="psm")
                    for n in range(NQ - 1, -1, -1):
                        for j in range(KP):
                            nc.tensor.matmul(
                                psm[:, n * 512:(n + 1) * 512],
                                xT[:, j, :, m * 128:(m + 1) * 128],
                                embT[:, j, :, n * 512:(n + 1) * 512],
                                start=(j == 0), stop=(j == KP - 1),
                                perf_mode=DR)
                    expt = work.tile([128, NQ * 512], bf16, tag="expt", bufs=4)
                    if m == MT - 1:
                        # last tile: let ACT accumulate the row-sum itself so
                        # the result exists the moment the exp retires
                        nc.scalar.activation(out=expt, in_=psm, func=AF.Exp,
                                             bias=biasM, scale=scl[:, m:m + 1],
                                             accum_out=ssb[:, m:m + 1])
                    else:
                        nc.scalar.activation(out=expt, in_=psm, func=AF.Exp,
                                             bias=biasM, scale=scl[:, m:m + 1])
                        # row-sum off the ACT engine (bf16 keeps DVE fast)
                        dummy = work.tile([128, NQ * 512], bf16, tag="dumm",
                                          bufs=2)
                        nc.vector.tensor_scalar(out=dummy, in0=expt,
                                                scalar1=1.0, scalar2=0.0,
                                                op0=OP.mult, op1=OP.add,
                                                accum_out=ssb[:, m:m + 1])
                    # target: gather exp(30cos - M) of window col widx_b from
                    # the exp OUTPUT (keeps the exp off the psm-reader chain);
                    # host recovers 30cos = ln(g) + M
                    scr = work.tile([128, U], f32, tag="scr")
                    nc.vector.scalar_tensor_tensor(
                        out=scr, in0=iotab, scalar=widx[:, m:m + 1],
                        in1=expt[:, 0:U], op0=OP.is_equal, op1=OP.mult,
                        accum_out=praw[:, m:m + 1])
                    if m == MT - 2:      # all but the last column go out early
                        nc.sync.dma_start(out=sume_d[:, 0:MT - 1],
                                          in_=ssb[:, 0:MT - 1])
                nc.sync.dma_start(out=tco_d, in_=praw)

            nc.sync.dma_start(out=sume_d[:, MT - 1:MT],
                              in_=ssb[:, MT - 1:MT])

    nc.compile()
    return nc


def _host_bookkeeping(labels, label_cq, header_cq):
    """Mirror the reference's integer-only queue-update semantics."""
    labels = np.asarray(labels).astype(np.int64)
    lab = np.asarray(label_cq).astype(np.int64).copy()
    h0 = int(np.asarray(header_cq))

    # jnp.unique(labels, size=U): sorted unique, padded with the minimum
    uq = np.unique(labels)
    if uq.size < U:
        uniq = np.concatenate([uq, np.full(U - uq.size, uq.min(), np.int64)])
    else:
        uniq = uq[:U]

    emb_src = np.full(Q, -1, np.int64)   # >=0: row u of uniq means; -1: original
    h = h0 % Q
    for u in range(U):
        y = uniq[u]
        m = lab == y
        i = int(np.argmax(m)) if m.any() else 0
        inval = bool(m.any()) and (i != h)
        emb_src[h] = u
        lab[h] = y
        if inval:
            lab[i] = IGNORE
        h = (h + 1) % Q

    good = lab != IGNORE
    goodidx = np.flatnonzero(good)
    gl = lab[goodidx]
    vals, first = np.unique(gl, return_index=True)
    pos = np.searchsorted(vals, labels)
    assert np.all(vals[np.clip(pos, 0, vals.size - 1)] == labels), \
        "batch label missing from queue"
    xe = goodidx[first[pos]]
    return uniq, emb_src, good, xe, h0


def _prepare(inputs, labels, emb_cq, label_cq, header_cq):
    """Host bookkeeping -> (M, per-core input maps, extras, correction)."""
    inputs = np.ascontiguousarray(np.asarray(inputs, np.float32))
    emb_cq = np.ascontiguousarray(np.asarray(emb_cq, np.float32))
    labels = np.asarray(labels)

    uniq, emb_src, good, xe, h0 = _host_bookkeeping(labels, label_cq,
                                                    header_cq)

    # safe upper bound for any logit: 30 * max row norm (+fp8 slack)
    max_nrm = float(np.sqrt((emb_cq.astype(np.float64) ** 2).sum(axis=1).max()))
    M = OIM_SCALAR * max(1.0, max_nrm) * 1.10

    w_idx = emb_src[xe].astype(np.float64)        # -1 for non-window targets
    extra = np.flatnonzero(w_idx < 0)             # handled on host (rare/none)

    # per-pid masked means -> normalized window embeddings (4 MFLOP)
    x64 = inputs.astype(np.float64)
    m_u = (uniq[:, None] == labels[None, :].astype(np.int64))
    uniq_emb = (m_u.astype(np.float64) @ x64) / m_u.sum(axis=1, keepdims=True)
    uniq_emb /= np.maximum(
        np.linalg.norm(uniq_emb, axis=1, keepdims=True), 1e-12)

    # rotate the queue so the window is slots [0, U) -> core 0, cols [0, U)
    rot = (h0 + np.arange(Q)) % Q
    emb_rot = emb_cq[rot].copy()
    good_rot = good[rot]
    src_rot = emb_src[rot]
    assert np.all(src_rot[:U] == np.arange(U)) and np.all(src_rot[U:] < 0)
    emb_rot[:U] = uniq_emb                        # the queue-window write
    # zero invalidated slots; host subtracts their exp(0-M) later
    zero_rows = np.zeros(Q, bool)
    zero_rows[U:] = ~good_rot[U:]
    emb_rot[zero_rows] = 0.0
    n_bad = int(zero_rows.sum())

    def dmajor8(a):  # [R, D] f32 -> [128, KP, 2, R] fp8 (DoubleRow pairs)
        r = a.shape[0]
        return np.ascontiguousarray(
            a.T.reshape(KP, 2, 128, r).transpose(2, 0, 1, 3).astype(FP8))

    # 30 / ||x_b|| (exp scale; folds input normalization)
    nrm = np.sqrt((x64 ** 2).sum(axis=1))
    sclv = OIM_SCALAR / np.maximum(nrm, 1e-12)

    # consts columns: scl | widx | iota | tick
    consts = np.concatenate([
        sclv.reshape(MT, 128).T.astype(np.float32),
        w_idx.reshape(MT, 128).T.astype(np.float32),
        np.broadcast_to(np.arange(U, dtype=np.float32), (128, U)),
        np.zeros((128, 4), np.float32),
    ], axis=1)

    base = {
        "xt8": dmajor8(inputs),
        "consts": np.ascontiguousarray(consts),
    }
    in_maps = []
    for c in range(N_CORES):
        in_maps.append({
            **base,
            "embt8": dmajor8(emb_rot[c * QS:(c + 1) * QS].astype(np.float32)),
        })
    return M, in_maps, extra, xe, n_bad


def _combine(res_list, M, extra, xe, n_bad, inputs, emb_cq):
    """Unshard / combine per-core partials into the scalar loss."""
    S = np.zeros(B, np.float64)
    for r in res_list:
        S += r["sume"].astype(np.float64).T.reshape(B)
    S -= n_bad * np.exp(-float(M))                # zeroed rows' exp(0 - M)
    g = res_list[0]["tco"].astype(np.float64).T.reshape(B)
    t30 = np.log(np.maximum(g, 1e-300)) + M       # 30*cos(target)

    if extra.size:  # targets pointing at original (non-window) queue rows
        xb = np.asarray(inputs, np.float64)[extra]
        xb /= np.maximum(np.linalg.norm(xb, axis=1, keepdims=True), 1e-12)
        eb = np.asarray(emb_cq, np.float64)[xe[extra]]
        t30[extra] = OIM_SCALAR * (xb * eb).sum(axis=1)

    loss = np.mean(M + np.log(S) - t30)
    return np.array(loss, dtype=np.float32)


def kernel(inputs, labels, emb_cq, label_cq, age_cq, header_cq):
    from concourse.bass_utils import run_bass_kernel_spmd

    M, in_maps, extra, xe, n_bad = _prepare(inputs, labels, emb_cq, label_cq,
                                            header_cq)

    key = round(M, 9)
    if key not in _PROG_CACHE:
        _PROG_CACHE[key] = _build_program(M)
    nc = _PROG_CACHE[key]

    res = run_bass_kernel_spmd(nc, in_maps, core_ids=list(range(N_CORES)))
    return _combine(res.results, M, extra, xe, n_bad, inputs, emb_cq)


# revision 25
# speedup vs baseline: 3.5525x; 1.0015x over previous
"""OIM loss with circular queue — Trainium2 Bass kernel (8 NeuronCores).

Strategy
--------
The output is a scalar:  loss = mean_b [ logsumexp_{q in good}(30*cos(x_b, e_q))
                                         - 30*cos(x_b, e_{xe_b}) ]
where e is the circular queue after the (sequential, data-dependent) update.

All O(B*D + Q*D) bookkeeping and reshaping runs on the host: the integer
queue update, the per-pid masked means + queue-window write (4 MFLOP), the
rotation of the queue so the window is always core 0 / cols [0,U), zeroing
of invalidated slots (their exact n_bad * exp(-M) contribution is
subtracted from the device sums), fp8 quantization, and the d-major
DoubleRow layouts.  The O(B*Q*D) = 68.7 GFLOP logits matmul and the
B*Q = 67M-element exp/log-sum-exp run on the 8 cores, tensor-parallel
over Q (2048 queue rows per core):

  per m-tile of 128 rows: 8 fp8 DoubleRow matmuls accumulate the [128,2048]
  logits into one 4-bank PSUM tile; ONE exp activation per tile applies the
  per-row scale 30/||x_b|| (input normalization folded into the activation
  scale) and bias -M; row-sums of the bf16 exp output run on DVE/GPSIMD
  (alternating, to stay off the ACT critical path); the target cosine is
  gathered from cols [0,256) of PSUM with a one-hot DVE op (the window
  holds every batch pid's embedding).

The host adds the 8 partial sums, applies the zero-row correction, takes
log and means.
"""

import os
import sys

import numpy as np

for _p in ("/opt/trn_rl_repo", "/root/.axon_site/_ro/trn_rl_repo"):
    if os.path.isdir(_p) and _p not in sys.path:
        sys.path.insert(0, _p)

import ml_dtypes

BF16 = ml_dtypes.bfloat16
FP8 = ml_dtypes.float8_e4m3

B, D, Q, U = 4096, 512, 16384, 256
N_CORES = 8
QS = Q // N_CORES          # queue rows per core
OIM_SCALAR = 30.0
IGNORE = -1
MT = B // 128              # 32 b-tiles
KD = D // 128              # 4 contraction chunks
KP = KD // 2               # 2 chunk pairs (DoubleRow)
NQ = QS // 512             # 4 matmul n-chunks per core

_PROG_CACHE = {}


def _build_program(M: float):
    """Emit + schedule + compile the (SPMD, identical on all cores) program."""
    import concourse.bacc as bacc
    import concourse.tile as tile
    from concourse import mybir
    from concourse.masks import make_identity

    f32 = mybir.dt.float32
    bf16 = mybir.dt.bfloat16
    fp8 = mybir.dt.float8e4
    AF = mybir.ActivationFunctionType
    OP = mybir.AluOpType
    DR = mybir.MatmulPerfMode.DoubleRow

    nc = bacc.Bacc("TRN2", target_bir_lowering=False, debug=False,
                   num_devices=N_CORES)

    # consts columns: scl | widx | iota | tick
    CW = MT + MT + U + 4
    xt_d = nc.dram_tensor("xt8", [128, KP, 2, B], fp8, kind="ExternalInput").ap()
    embt_d = nc.dram_tensor("embt8", [128, KP, 2, QS], fp8,
                            kind="ExternalInput").ap()
    consts_d = nc.dram_tensor("consts", [128, CW], f32,
                              kind="ExternalInput").ap()
    sume_d = nc.dram_tensor("sume", [128, MT], f32, kind="ExternalOutput").ap()
    tco_d = nc.dram_tensor("tco", [128, MT], f32, kind="ExternalOutput").ap()
    tock_d = nc.dram_tensor("tock", [128, 4], f32, kind="ExternalOutput").ap()

    with tile.TileContext(nc) as tc:
        with (
            tc.tile_pool(name="singles", bufs=1) as singles,
            tc.tile_pool(name="work", bufs=4) as work,
            tc.tile_pool(name="small", bufs=4) as small,
        ):
            # one merged consts DMA:  scl | widx | iota | tick
            consts = singles.tile([128, CW], f32)
            nc.sync.dma_start(out=consts, in_=consts_d)
            scl = consts[:, 0:MT]
            widx = consts[:, MT:2 * MT]
            iotab = consts[:, 2 * MT:2 * MT + U]
            tickt = consts[:, 2 * MT + U:CW]

            biasM = singles.tile([128, 1], f32)
            nc.vector.memset(biasM, -M)
            identb = singles.tile([128, 128], bf16)
            make_identity(nc, identb)

            # preload the Exp table off the critical path (the only ACT func)
            warm = small.tile([128, 1], f32, tag="warm")
            nc.vector.memset(warm, 1.0)
            warm2 = small.tile([128, 1], f32, tag="warm2")
            nc.scalar.activation(out=warm2, in_=warm, func=AF.Exp)

            ssb = singles.tile([128, MT], f32)    # sum-exp out collector
            praw = singles.tile([128, MT], f32)   # exp(target logit - M) out
            embT = singles.tile([128, KP, 2, QS], fp8)
            xT = singles.tile([128, KP, 2, B], fp8)

            # tick/tock passthrough early (no compute dependency)
            nc.sync.dma_start(out=tock_d, in_=tickt)

            # input streams in consumption order (n runs 3..0, m runs 0..31):
            # fine chunks up front so the first PSUM tile completes early,
            # coarse chunks after
            nc.sync.dma_start(out=embT[:, :, :, 3 * QS // 4:QS],
                              in_=embt_d[:, :, :, 3 * QS // 4:QS])
            nc.sync.dma_start(out=xT[:, :, :, 0:B // 8],
                              in_=xt_d[:, :, :, 0:B // 8])
            for c in (2, 1, 0):
                nc.sync.dma_start(
                    out=embT[:, :, :, c * QS // 4:(c + 1) * QS // 4],
                    in_=embt_d[:, :, :, c * QS // 4:(c + 1) * QS // 4])
            for c in range(1, 8):
                nc.sync.dma_start(out=xT[:, :, :, c * B // 8:(c + 1) * B // 8],
                                  in_=xt_d[:, :, :, c * B // 8:(c + 1) * B // 8])

            # spin the PE p-state up with throwaway transposes so the first
            # logits matmuls run at full clock the moment their DMAs land
            with tc.tile_pool(name="psum_w", bufs=1, space="PSUM") as psum_w:
                wps = psum_w.tile([128, 128], bf16, tag="wps")
                for _ in range(34):
                    nc.tensor.transpose(wps, identb, identb)

            # ---------------- logits + exp + row sums ----------------
            with tc.tile_pool(name="psum_m", bufs=2, space="PSUM") as psum_m:
                for m in range(MT):
                    psm = psum_m.tile([128, NQ * 512], f32, tag="psm")
                    for n in range(NQ - 1, -1, -1):
                        for j in range(KP):
                            nc.tensor.matmul(
                                psm[:, n * 512:(n + 1) * 512],
                                xT[:, j, :, m * 128:(m + 1) * 128],
                                embT[:, j, :, n * 512:(n + 1) * 512],
                                start=(j == 0), stop=(j == KP - 1),
                                perf_mode=DR)
                    # target: gather the raw dot of window col widx_b from
                    # PSUM; the host applies the 30/||x_b|| scale
                    scr = work.tile([128, U], f32, tag="scr")
                    nc.vector.scalar_tensor_tensor(
                        out=scr, in0=iotab, scalar=widx[:, m:m + 1],
                        in1=psm[:, 0:U], op0=OP.is_equal, op1=OP.mult,
                        accum_out=praw[:, m:m + 1])
                    expt = work.tile([128, NQ * 512], bf16, tag="expt", bufs=4)
                    if m == MT - 1:
                        # last tile: let ACT accumulate the row-sum itself so
                        # the result exists the moment the exp retires
                        nc.scalar.activation(out=expt, in_=psm, func=AF.Exp,
                                             bias=biasM, scale=scl[:, m:m + 1],
                                             accum_out=ssb[:, m:m + 1])
                        nc.sync.dma_start(out=tco_d, in_=praw)
                    else:
                        nc.scalar.activation(out=expt, in_=psm, func=AF.Exp,
                                             bias=biasM, scale=scl[:, m:m + 1])
                        # row-sum off the ACT engine (bf16 keeps DVE fast)
                        dummy = work.tile([128, NQ * 512], bf16, tag="dumm",
                                          bufs=2)
                        nc.vector.tensor_scalar(out=dummy, in0=expt,
                                                scalar1=1.0, scalar2=0.0,
                                                op0=OP.mult, op1=OP.add,
                                                accum_out=ssb[:, m:m + 1])
                    if m == MT - 2:      # all but the last column go out early
                        nc.sync.dma_start(out=sume_d[:, 0:MT - 1],
                                          in_=ssb[:, 0:MT - 1])

            nc.sync.dma_start(out=sume_d[:, MT - 1:MT],
                              in_=ssb[:, MT - 1:MT])

    nc.compile()
    return nc


def _host_bookkeeping(labels, label_cq, header_cq):
    """Mirror the reference's integer-only queue-update semantics."""
    labels = np.asarray(labels).astype(np.int64)
    lab = np.asarray(label_cq).astype(np.int64).copy()
    h0 = int(np.asarray(header_cq))

    # jnp.unique(labels, size=U): sorted unique, padded with the minimum
    uq = np.unique(labels)
    if uq.size < U:
        uniq = np.concatenate([uq, np.full(U - uq.size, uq.min(), np.int64)])
    else:
        uniq = uq[:U]

    emb_src = np.full(Q, -1, np.int64)   # >=0: row u of uniq means; -1: original
    h = h0 % Q
    for u in range(U):
        y = uniq[u]
        m = lab == y
        i = int(np.argmax(m)) if m.any() else 0
        inval = bool(m.any()) and (i != h)
        emb_src[h] = u
        lab[h] = y
        if inval:
            lab[i] = IGNORE
        h = (h + 1) % Q

    good = lab != IGNORE
    goodidx = np.flatnonzero(good)
    gl = lab[goodidx]
    vals, first = np.unique(gl, return_index=True)
    pos = np.searchsorted(vals, labels)
    assert np.all(vals[np.clip(pos, 0, vals.size - 1)] == labels), \
        "batch label missing from queue"
    xe = goodidx[first[pos]]
    return uniq, emb_src, good, xe, h0


def _prepare(inputs, labels, emb_cq, label_cq, header_cq):
    """Host bookkeeping -> (M, per-core input maps, extras, correction)."""
    inputs = np.ascontiguousarray(np.asarray(inputs, np.float32))
    emb_cq = np.ascontiguousarray(np.asarray(emb_cq, np.float32))
    labels = np.asarray(labels)

    uniq, emb_src, good, xe, h0 = _host_bookkeeping(labels, label_cq,
                                                    header_cq)

    # safe upper bound for any logit: 30 * max row norm (+fp8 slack)
    max_nrm = float(np.sqrt((emb_cq.astype(np.float64) ** 2).sum(axis=1).max()))
    M = OIM_SCALAR * max(1.0, max_nrm) * 1.10

    w_idx = emb_src[xe].astype(np.float64)        # -1 for non-window targets
    extra = np.flatnonzero(w_idx < 0)             # handled on host (rare/none)

    # per-pid masked means -> normalized window embeddings (4 MFLOP)
    x64 = inputs.astype(np.float64)
    m_u = (uniq[:, None] == labels[None, :].astype(np.int64))
    uniq_emb = (m_u.astype(np.float64) @ x64) / m_u.sum(axis=1, keepdims=True)
    uniq_emb /= np.maximum(
        np.linalg.norm(uniq_emb, axis=1, keepdims=True), 1e-12)

    # rotate the queue so the window is slots [0, U) -> core 0, cols [0, U)
    rot = (h0 + np.arange(Q)) % Q
    emb_rot = emb_cq[rot].copy()
    good_rot = good[rot]
    src_rot = emb_src[rot]
    assert np.all(src_rot[:U] == np.arange(U)) and np.all(src_rot[U:] < 0)
    emb_rot[:U] = uniq_emb                        # the queue-window write
    # zero invalidated slots; host subtracts their exp(0-M) later
    zero_rows = np.zeros(Q, bool)
    zero_rows[U:] = ~good_rot[U:]
    emb_rot[zero_rows] = 0.0
    n_bad = int(zero_rows.sum())

    def dmajor8(a):  # [R, D] f32 -> [128, KP, 2, R] fp8 (DoubleRow pairs)
        r = a.shape[0]
        return np.ascontiguousarray(
            a.T.reshape(KP, 2, 128, r).transpose(2, 0, 1, 3).astype(FP8))

    # 30 / ||x_b|| (exp scale; folds input normalization)
    nrm = np.sqrt((x64 ** 2).sum(axis=1))
    sclv = OIM_SCALAR / np.maximum(nrm, 1e-12)

    # consts columns: scl | widx | iota | tick
    consts = np.concatenate([
        sclv.reshape(MT, 128).T.astype(np.float32),
        w_idx.reshape(MT, 128).T.astype(np.float32),
        np.broadcast_to(np.arange(U, dtype=np.float32), (128, U)),
        np.zeros((128, 4), np.float32),
    ], axis=1)

    base = {
        "xt8": dmajor8(inputs),
        "consts": np.ascontiguousarray(consts),
    }
    in_maps = []
    for c in range(N_CORES):
        in_maps.append({
            **base,
            "embt8": dmajor8(emb_rot[c * QS:(c + 1) * QS].astype(np.float32)),
        })
    return M, in_maps, extra, xe, n_bad, sclv


def _combine(res_list, M, extra, xe, n_bad, sclv, inputs, emb_cq):
    """Unshard / combine per-core partials into the scalar loss."""
    S = np.zeros(B, np.float64)
    for r in res_list:
        S += r["sume"].astype(np.float64).T.reshape(B)
    S -= n_bad * np.exp(-float(M))                # zeroed rows' exp(0 - M)
    t30 = res_list[0]["tco"].astype(np.float64).T.reshape(B) * sclv

    if extra.size:  # targets pointing at original (non-window) queue rows
        xb = np.asarray(inputs, np.float64)[extra]
        xb /= np.maximum(np.linalg.norm(xb, axis=1, keepdims=True), 1e-12)
        eb = np.asarray(emb_cq, np.float64)[xe[extra]]
        t30[extra] = OIM_SCALAR * (xb * eb).sum(axis=1)

    loss = np.mean(M + np.log(S) - t30)
    return np.array(loss, dtype=np.float32)


def kernel(inputs, labels, emb_cq, label_cq, age_cq, header_cq):
    from concourse.bass_utils import run_bass_kernel_spmd

    M, in_maps, extra, xe, n_bad, sclv = _prepare(
        inputs, labels, emb_cq, label_cq, header_cq)

    key = round(M, 9)
    if key not in _PROG_CACHE:
        _PROG_CACHE[key] = _build_program(M)
    nc = _PROG_CACHE[key]

    res = run_bass_kernel_spmd(nc, in_maps, core_ids=list(range(N_CORES)))
    return _combine(res.results, M, extra, xe, n_bad, sclv, inputs, emb_cq)
